# revision 4
# baseline (speedup 1.0000x reference)
"""Trainium2 Bass kernel for masked 15-bin ECE (v3: pair-batched probes).

Same math as v2 (see kernel.py docstring): ECE = sum_b |S_b|/total,
S_b = L_{b-1} - L_b, L_t = A_t - 4*B_t with
  A_t = sum relu(z - (4t+2)),  z = 4*bin + d (fp16 ceil-trick binning)
  B_t = sum relu(u' - (1024.5+t)), u' = 1024 + bin
DVE tensor_scalar accum probes compute M = sum max(stream, thresh) =
target + thresh*N (N = element count, exact); ACT activations accumulate
relu sums directly.

v3 batches the 30 probes per PAIR of [128, 4096] tiles: z and u' are
written into [128, 2, 4096] rings, and each probe instruction scans the
8192-element free range, halving per-instruction fixed overheads (DVE
58-cycle SBUF access, ACT 222-cycle access + 187ns accumulator read).
"""

import os
import sys

for _p in ("/opt/trn_rl_repo",):
    if _p not in sys.path and os.path.isdir(_p):
        sys.path.insert(0, _p)

import numpy as np

import concourse.bacc as bacc
import concourse.mybir as mybir
import concourse.tile as tile
from concourse.bass_utils import run_bass_kernel_spmd

N_CORES = 8
N_BINS = 15
FULL_ROWS = 8192
COLS = 4096
ROWS = FULL_ROWS // N_CORES   # 1024 rows per core
FREE = 4096
P = 128
OFF = 1024.0
LAST_EXEC_TIME_NS = None
LAST_RESULTS = None
_CACHE = {}

# Per-pair ACT B-probe assignment (list of sets, len n_pairs). Heavier ACT
# load early, lighter on the last pair so both engines finish together.
_ACT_SCHED = [set(range(9)) for _ in range(4)]


def _build_program(rows=ROWS, cols=COLS, free=FREE, num_devices=N_CORES):
    n_r = rows // P
    n_c = cols // free
    n_tiles = n_r * n_c
    assert n_tiles % 2 == 0
    n_pairs = n_tiles // 2
    assert len(_ACT_SCHED) == n_pairs

    nc = bacc.Bacc(
        "TRN2", target_bir_lowering=False, debug=False, num_devices=num_devices
    )

    f32 = mybir.dt.float32
    f16 = mybir.dt.float16

    X = nc.dram_tensor("x", [rows, 4, cols], f16, kind="ExternalInput").ap()
    outA = nc.dram_tensor("pA", [P, n_pairs * N_BINS], f32, kind="ExternalOutput").ap()
    outB = nc.dram_tensor("pB", [P, n_pairs * N_BINS], f32, kind="ExternalOutput").ap()

    Alu = mybir.AluOpType
    Act = mybir.ActivationFunctionType

    with tile.TileContext(nc) as tc:
        with (
            tc.tile_pool(name="inp", bufs=2) as inp,
            tc.tile_pool(name="work", bufs=1) as work,
            tc.tile_pool(name="ring", bufs=2) as ring,
            tc.tile_pool(name="ringz", bufs=1) as ringz,
            tc.tile_pool(name="scr", bufs=1) as scrp,
            tc.tile_pool(name="stage", bufs=1) as stage,
        ):
            LA = stage.tile([P, n_pairs * N_BINS], f32, tag="LA")
            LB = stage.tile([P, n_pairs * N_BINS], f32, tag="LB")
            scrD = scrp.tile([P, 2, free], f16, tag="scrD")
            scrA = scrp.tile([P, 2, free], f16, tag="scrA")
            biasB = {}
            all_act = set().union(*_ACT_SCHED)
            for t in sorted(all_act):
                bb = stage.tile([P, 1], f32, tag=f"bb{t}")
                nc.vector.memset(bb[:], -(OFF + 0.5 + t))
                biasB[t] = bb

            for pr in range(n_pairs):
                upR = ring.tile([P, 2, free], f16, tag="upR")
                # zR/u4R are produced and consumed by DVE only (in-order
                # engine), so a single buffer can't stall cross-engine.
                zR = ringz.tile([P, 2, free], f16, tag="zR")
                u4R = ringz.tile([P, 2, free], f16, tag="u4R")
                for half in range(2):
                    it = pr * 2 + half
                    r0 = (it // n_c) * P
                    c0 = (it % n_c) * free

                    xt = inp.tile([P, 4, free], f16, tag="x")
                    cm = work.tile([P, free], f16, tag="cm")
                    corr = work.tile([P, free], f16, tag="corr")
                    d = work.tile([P, free], f16, tag="d")

                    n_chunks = 1
                    cw = free // n_chunks
                    for q in range(n_chunks):
                        cs = slice(q * cw, (q + 1) * cw)
                        nc.sync.dma_start(
                            xt[:, :, cs], X[r0 : r0 + P, :, c0 + q * cw : c0 + (q + 1) * cw]
                        )
                    for q in range(n_chunks):
                        cs = slice(q * cw, (q + 1) * cw)
                        c16 = xt[:, 0, cs]
                        p16 = xt[:, 1, cs]
                        t16 = xt[:, 2, cs]
                        m16 = xt[:, 3, cs]
                        nc.vector.tensor_tensor(cm[:, cs], c16, m16, Alu.mult)
                        nc.vector.tensor_tensor(corr[:, cs], p16, t16, Alu.is_equal)
                        nc.vector.tensor_scalar(
                            upR[:, half, cs], cm[:, cs], 15.0, OFF + 0.5,
                            Alu.mult, Alu.add,
                        )
                        nc.vector.tensor_tensor(d[:, cs], cm[:, cs], corr[:, cs], Alu.subtract)
                        # u4 = (u' - 1024)*4 = 4*bin, small-magnitude B stream
                        nc.vector.tensor_scalar(
                            u4R[:, half, cs], upR[:, half, cs], OFF, 4.0,
                            Alu.subtract, Alu.mult,
                        )
                        nc.vector.tensor_tensor(
                            zR[:, half, cs], u4R[:, half, cs], d[:, cs], Alu.add
                        )

                act_set = _ACT_SCHED[pr]
                for t in range(N_BINS):
                    colA = slice(pr * N_BINS + t, pr * N_BINS + t + 1)
                    colB = slice(pr * N_BINS + t, pr * N_BINS + t + 1)
                    # A-probe over both halves: accum = sum max(z, 4t+2)
                    nc.vector.tensor_scalar(
                        scrD[:], zR[:], 4.0 * t + 2.0, 0.0,
                        Alu.max, Alu.add, accum_out=LA[:, colA],
                    )
                    if t in act_set:
                        nc.scalar.activation(
                            scrA[:], upR[:], Act.Relu,
                            bias=biasB[t][:], accum_out=LB[:, colB],
                        )
                    else:
                        # same scalar as the A-probe: L_t = M_t - B'_t with
                        # the (4t+2)*N offsets cancelling exactly, and the
                        # u4 stream keeps f32 accumulation well-conditioned
                        nc.vector.tensor_scalar(
                            scrD[:], u4R[:], 4.0 * t + 2.0, 0.0,
                            Alu.max, Alu.add, accum_out=LB[:, colB],
                        )

                # stream this pair's accumulator columns out while the next
                # pair computes, so the final drain only waits on pair n-1
                cols = slice(pr * N_BINS, (pr + 1) * N_BINS)
                nc.sync.dma_start(outA[:, cols], LA[:, cols])
                nc.sync.dma_start(outB[:, cols], LB[:, cols])

    nc.compile()
    return nc, n_tiles


def _get_program():
    if "prog" not in _CACHE:
        _CACHE["prog"] = _build_program()
    return _CACHE["prog"]


def _combine(partA_list, partB_list, total):
    if total == 0.0:
        return np.float32(0.0)
    # Per (pair, t): DVE B'-probes share the A-probe scalar so
    # L contribution = M - B' directly; ACT relu-probes need the
    # (4t+2)*N offset removed from M and contribute -4*sum(relu).
    L = np.zeros(N_BINS, dtype=np.float64)
    n_pairs = len(_ACT_SCHED)
    n_elem_pair = float(P * 2 * FREE)
    for pa, pb in zip(partA_list, partB_list):
        pa = np.asarray(pa).astype(np.float64).reshape(P, n_pairs, N_BINS)
        pb = np.asarray(pb).astype(np.float64).reshape(P, n_pairs, N_BINS)
        for pr in range(n_pairs):
            act_set = _ACT_SCHED[pr]
            for t in range(N_BINS):
                M = pa[:, pr, t].sum()
                bv = pb[:, pr, t].sum()
                if t in act_set:
                    L[t] += M - (4.0 * t + 2.0) * n_elem_pair - 4.0 * bv
                else:
                    L[t] += M - bv
    S = L.copy()
    S[:-1] -= L[1:]
    return np.float32(np.abs(S).sum() / total)


def kernel(confidences, predictions, targets, mask):
    global LAST_EXEC_TIME_NS, LAST_RESULTS
    nc, n_tiles = _get_program()

    conf = np.asarray(confidences, dtype=np.float32)
    pred = np.asarray(predictions)
    targ = np.asarray(targets)
    msk = np.asarray(mask)
    assert conf.shape == (FULL_ROWS, COLS)

    X = np.empty((FULL_ROWS, 4, COLS), dtype=np.float16)
    X[:, 0] = conf.astype(np.float16)
    X[:, 1] = pred.astype(np.float16)
    X[:, 2] = targ.astype(np.float16)
    X[:, 3] = msk.astype(np.float16)

    in_maps = []
    for i in range(N_CORES):
        sl = slice(i * ROWS, (i + 1) * ROWS)
        in_maps.append({"x": X[sl]})

    trace = bool(int(os.environ.get("ECE_TRACE", "0")))
    res = run_bass_kernel_spmd(nc, in_maps, list(range(N_CORES)), trace=trace)
    LAST_EXEC_TIME_NS = res.exec_time_ns
    LAST_RESULTS = res

    total = float(np.asarray(msk).sum(dtype=np.int64))
    return _combine(
        [res.results[i]["pA"] for i in range(N_CORES)],
        [res.results[i]["pB"] for i in range(N_CORES)],
        total,
    )


# revision 5
# speedup vs baseline: 1.0149x; 1.0149x over previous
"""Trainium2 Bass kernel for masked 15-bin Expected Calibration Error.

Contract: kernel(**full_inputs) -> full scalar output. Row-shards the four
[8192, 4096] tensors across 8 NeuronCores (1024 rows each). Host staging is
dtype/layout only: fp16 casts packed into one [rows, 4, cols] array per core
(conf, pred, targ, mask) plus the trivial total = sum(mask) reduction.

Math: ECE = sum_b |S_b| / total with S_b = sum_{bin==b} mask*(conf - corr),
via cumulative sums L_t = sum_{bin>t} mask*(conf - corr) = A_t - 4*B_t:
  u' = fp16(15*c*m + 1024.5)   exact 1024 + ceil(15*c*m): fp16 ulp is 1 on
       [1024, 2048] so the store rounds to the bin integer; the only tie
       (c*m == 0 -> 1024.5) rounds half-to-even DOWN to bin 0 = excluded,
       matching the reference's c > 0 gate and the mask.
  u4 = (u' - 1024)*4           exact 4*bin, small-magnitude count stream
  z  = u4 + d, d = c*m - corr  (|d| <= 1 < 2 margin at thresholds 4t+2)
  A_t = sum relu(z - (4t+2)),  B_t = sum relu(u4 - (4t+2)) / 4

Engine mapping (DVE tensor_scalar = 4x perf mode with 2-byte operands;
tensor_tensor = 2x; ACT = 1x; tensor_scalar's accum_out makes op1 the
REDUCTION op, so probes compute M = sum max(stream, 4t+2) and the A/B
offsets (4t+2)*N cancel exactly in L_t = M_t(z) - M_t(u4)):
  per [128, 4096] tile: DVE preps = 3 TT (c*m, corr, d) + TS (u'), TS (u4),
  TT (z); per PAIR of tiles (probes scan [128, 2, 4096] rings to amortize
  fixed overheads): DVE 15 A-probes + 6 B-probes @4x, ACT 9 B-probes
  (Relu + bias + accumulator), which saturates the ACT pair window.
Accumulator columns stream out per pair; the host combines in f64.
"""

import os
import sys

for _p in ("/opt/trn_rl_repo",):
    if _p not in sys.path and os.path.isdir(_p):
        sys.path.insert(0, _p)

import numpy as np

import concourse.bacc as bacc
import concourse.mybir as mybir
import concourse.tile as tile
from concourse.bass_utils import run_bass_kernel_spmd

N_CORES = 8
N_BINS = 15
FULL_ROWS = 8192
COLS = 4096
ROWS = FULL_ROWS // N_CORES   # 1024 rows per core
FREE = 4096
P = 128
OFF = 1024.0
LAST_EXEC_TIME_NS = None
LAST_RESULTS = None
_CACHE = {}

# Per-pair ACT B-probe assignment (list of sets, len n_pairs). Heavier ACT
# load early, lighter on the last pair so both engines finish together.
_ACT_SCHED = [set(range(9)) for _ in range(4)]


def _build_program(rows=ROWS, cols=COLS, free=FREE, num_devices=N_CORES):
    n_r = rows // P
    n_c = cols // free
    n_tiles = n_r * n_c
    assert n_tiles % 2 == 0
    n_pairs = n_tiles // 2
    assert len(_ACT_SCHED) == n_pairs

    nc = bacc.Bacc(
        "TRN2", target_bir_lowering=False, debug=False, num_devices=num_devices
    )

    f32 = mybir.dt.float32
    f16 = mybir.dt.float16

    X = nc.dram_tensor("x", [rows, 4, cols], f16, kind="ExternalInput").ap()
    outA = nc.dram_tensor("pA", [P, n_pairs * N_BINS], f32, kind="ExternalOutput").ap()
    outB = nc.dram_tensor("pB", [P, n_pairs * N_BINS], f32, kind="ExternalOutput").ap()

    Alu = mybir.AluOpType
    Act = mybir.ActivationFunctionType

    with tile.TileContext(nc) as tc:
        with (
            tc.tile_pool(name="inp", bufs=2) as inp,
            tc.tile_pool(name="work", bufs=1) as work,
            tc.tile_pool(name="ring", bufs=2) as ring,
            tc.tile_pool(name="ringz", bufs=1) as ringz,
            tc.tile_pool(name="scr", bufs=1) as scrp,
            tc.tile_pool(name="stage", bufs=1) as stage,
        ):
            LA = stage.tile([P, n_pairs * N_BINS], f32, tag="LA")
            LB = stage.tile([P, n_pairs * N_BINS], f32, tag="LB")
            scrD = scrp.tile([P, 2, free], f16, tag="scrD")
            scrA = scrp.tile([P, 2, free], f16, tag="scrA")
            biasB = {}
            all_act = set().union(*_ACT_SCHED)
            for t in sorted(all_act):
                bb = stage.tile([P, 1], f32, tag=f"bb{t}")
                nc.vector.memset(bb[:], -(OFF + 0.5 + t))
                biasB[t] = bb

            for pr in range(n_pairs):
                upR = ring.tile([P, 2, free], f16, tag="upR")
                # zR/u4R are produced and consumed by DVE only (in-order
                # engine), so a single buffer can't stall cross-engine.
                zR = ringz.tile([P, 2, free], f16, tag="zR")
                u4R = ringz.tile([P, 2, free], f16, tag="u4R")
                for half in range(2):
                    it = pr * 2 + half
                    r0 = (it // n_c) * P
                    c0 = (it % n_c) * free

                    xt = inp.tile([P, 4, free], f16, tag="x")
                    cm = work.tile([P, free], f16, tag="cm")
                    corr = work.tile([P, free], f16, tag="corr")
                    d = work.tile([P, free], f16, tag="d")

                    n_chunks = 1
                    cw = free // n_chunks
                    for q in range(n_chunks):
                        cs = slice(q * cw, (q + 1) * cw)
                        nc.sync.dma_start(
                            xt[:, :, cs], X[r0 : r0 + P, :, c0 + q * cw : c0 + (q + 1) * cw]
                        )
                    for q in range(n_chunks):
                        cs = slice(q * cw, (q + 1) * cw)
                        c16 = xt[:, 0, cs]
                        p16 = xt[:, 1, cs]
                        t16 = xt[:, 2, cs]
                        m16 = xt[:, 3, cs]
                        nc.vector.tensor_tensor(cm[:, cs], c16, m16, Alu.mult)
                        nc.vector.tensor_tensor(corr[:, cs], p16, t16, Alu.is_equal)
                        nc.vector.tensor_scalar(
                            upR[:, half, cs], cm[:, cs], 15.0, OFF + 0.5,
                            Alu.mult, Alu.add,
                        )
                        nc.vector.tensor_tensor(d[:, cs], cm[:, cs], corr[:, cs], Alu.subtract)
                        # u4 = (u' - 1024)*4 = 4*bin, small-magnitude B stream
                        nc.vector.tensor_scalar(
                            u4R[:, half, cs], upR[:, half, cs], OFF, 4.0,
                            Alu.subtract, Alu.mult,
                        )
                        nc.vector.tensor_tensor(
                            zR[:, half, cs], u4R[:, half, cs], d[:, cs], Alu.add
                        )

                act_set = _ACT_SCHED[pr]
                for t in range(N_BINS):
                    colA = slice(pr * N_BINS + t, pr * N_BINS + t + 1)
                    colB = slice(pr * N_BINS + t, pr * N_BINS + t + 1)
                    # A-probe over both halves: accum = sum max(z, 4t+2)
                    nc.vector.tensor_scalar(
                        scrD[:], zR[:], 4.0 * t + 2.0, 0.0,
                        Alu.max, Alu.add, accum_out=LA[:, colA],
                    )
                    if t in act_set:
                        nc.scalar.activation(
                            scrA[:], upR[:], Act.Relu,
                            bias=biasB[t][:], accum_out=LB[:, colB],
                        )
                    else:
                        # same scalar as the A-probe: L_t = M_t - B'_t with
                        # the (4t+2)*N offsets cancelling exactly, and the
                        # u4 stream keeps f32 accumulation well-conditioned
                        nc.vector.tensor_scalar(
                            scrD[:], u4R[:], 4.0 * t + 2.0, 0.0,
                            Alu.max, Alu.add, accum_out=LB[:, colB],
                        )

                # stream this pair's accumulator columns out while the next
                # pair computes, so the final drain only waits on pair n-1
                cols = slice(pr * N_BINS, (pr + 1) * N_BINS)
                nc.sync.dma_start(outA[:, cols], LA[:, cols])
                nc.sync.dma_start(outB[:, cols], LB[:, cols])

    nc.compile()
    return nc, n_tiles


def _get_program():
    if "prog" not in _CACHE:
        _CACHE["prog"] = _build_program()
    return _CACHE["prog"]


def _combine(partA_list, partB_list, total):
    if total == 0.0:
        return np.float32(0.0)
    # Per (pair, t): DVE B'-probes share the A-probe scalar so
    # L contribution = M - B' directly; ACT relu-probes need the
    # (4t+2)*N offset removed from M and contribute -4*sum(relu).
    L = np.zeros(N_BINS, dtype=np.float64)
    n_pairs = len(_ACT_SCHED)
    n_elem_pair = float(P * 2 * FREE)
    for pa, pb in zip(partA_list, partB_list):
        pa = np.asarray(pa).astype(np.float64).reshape(P, n_pairs, N_BINS)
        pb = np.asarray(pb).astype(np.float64).reshape(P, n_pairs, N_BINS)
        for pr in range(n_pairs):
            act_set = _ACT_SCHED[pr]
            for t in range(N_BINS):
                M = pa[:, pr, t].sum()
                bv = pb[:, pr, t].sum()
                if t in act_set:
                    L[t] += M - (4.0 * t + 2.0) * n_elem_pair - 4.0 * bv
                else:
                    L[t] += M - bv
    S = L.copy()
    S[:-1] -= L[1:]
    return np.float32(np.abs(S).sum() / total)


def kernel(confidences, predictions, targets, mask):
    global LAST_EXEC_TIME_NS, LAST_RESULTS
    nc, n_tiles = _get_program()

    conf = np.asarray(confidences, dtype=np.float32)
    pred = np.asarray(predictions)
    targ = np.asarray(targets)
    msk = np.asarray(mask)
    assert conf.shape == (FULL_ROWS, COLS)

    X = np.empty((FULL_ROWS, 4, COLS), dtype=np.float16)
    X[:, 0] = conf.astype(np.float16)
    X[:, 1] = pred.astype(np.float16)
    X[:, 2] = targ.astype(np.float16)
    X[:, 3] = msk.astype(np.float16)

    in_maps = []
    for i in range(N_CORES):
        sl = slice(i * ROWS, (i + 1) * ROWS)
        in_maps.append({"x": X[sl]})

    trace = bool(int(os.environ.get("ECE_TRACE", "0")))
    res = run_bass_kernel_spmd(nc, in_maps, list(range(N_CORES)), trace=trace)
    LAST_EXEC_TIME_NS = res.exec_time_ns
    LAST_RESULTS = res

    total = float(np.asarray(msk).sum(dtype=np.int64))
    return _combine(
        [res.results[i]["pA"] for i in range(N_CORES)],
        [res.results[i]["pB"] for i in range(N_CORES)],
        total,
    )


# revision 6
# speedup vs baseline: 1.0178x; 1.0029x over previous
"""Trainium2 Bass kernel for masked 15-bin Expected Calibration Error.

Contract: kernel(**full_inputs) -> full scalar output. Row-shards the four
[8192, 4096] tensors across 8 NeuronCores (1024 rows each). Host staging is
dtype/layout only: fp16 casts packed into one [rows, 4, cols] array per core
ordered (conf, mask, pred, targ), plus the trivial total = sum(mask).

Math: ECE = sum_b |S_b| / total with S_b = sum_{bin==b} mask*(conf - corr),
via cumulative sums L_t = sum_{bin>t} mask*(conf - corr):
  u' = fp16(15*c*m + 1024.5)   exact 1024 + ceil(15*c*m): fp16 ulp is 1 on
       [1024, 2048] so the store rounds to the bin integer; the only tie
       (c*m == 0 -> 1024.5) rounds half-to-even DOWN to bin 0 = excluded,
       matching the reference's c > 0 gate and the mask.
  u4 = (u' - 1024)*4           exact 4*bin, small-magnitude count stream
  z  = u4 + d, d = c*m - corr  (|d| <= 1 < 2 margin at thresholds 4t+2)
  L_t = sum max(z, 4t+2) - sum max(u4, 4t+2)   (offsets cancel exactly)

Engine mapping (DVE tensor_scalar = 4x perf mode with 2-byte SBUF operands;
tensor_tensor = 2x; ACT = 1x; tensor_scalar's accum_out makes op1 the
REDUCTION op, hence the max-probe form):
  per [128, 4096] tile: DVE preps = TT c*m, TS u', TT corr, TT d, TS u4,
  TT z; per PAIR of tiles (probes scan [128, 2, 4096] rings to amortize
  fixed overheads): DVE 15 z-probes + 6 u4-probes @4x, ACT 9 relu+bias+
  accumulate probes on u', which saturates the ACT pair window.
Each tile's DMA lands (c,m) before (p,t) so cm/u' start after a half-tile
transfer; accumulator columns stream out per pair; host combines in f64.
"""

import os
import sys

for _p in ("/opt/trn_rl_repo",):
    if _p not in sys.path and os.path.isdir(_p):
        sys.path.insert(0, _p)

import numpy as np

import concourse.bacc as bacc
import concourse.mybir as mybir
import concourse.tile as tile
from concourse.bass_utils import run_bass_kernel_spmd

N_CORES = 8
N_BINS = 15
FULL_ROWS = 8192
COLS = 4096
ROWS = FULL_ROWS // N_CORES   # 1024 rows per core
FREE = 4096
P = 128
OFF = 1024.0
LAST_EXEC_TIME_NS = None
LAST_RESULTS = None
_CACHE = {}

# Per-pair ACT B-probe assignment (list of sets, len n_pairs). Heavier ACT
# load early, lighter on the last pair so both engines finish together.
_ACT_SCHED = [set(range(9)) for _ in range(4)]


def _build_program(rows=ROWS, cols=COLS, free=FREE, num_devices=N_CORES):
    n_r = rows // P
    n_c = cols // free
    n_tiles = n_r * n_c
    assert n_tiles % 2 == 0
    n_pairs = n_tiles // 2
    assert len(_ACT_SCHED) == n_pairs

    nc = bacc.Bacc(
        "TRN2", target_bir_lowering=False, debug=False, num_devices=num_devices
    )

    f32 = mybir.dt.float32
    f16 = mybir.dt.float16

    X = nc.dram_tensor("x", [rows, 4, cols], f16, kind="ExternalInput").ap()
    outA = nc.dram_tensor("pA", [P, n_pairs * N_BINS], f32, kind="ExternalOutput").ap()
    outB = nc.dram_tensor("pB", [P, n_pairs * N_BINS], f32, kind="ExternalOutput").ap()

    Alu = mybir.AluOpType
    Act = mybir.ActivationFunctionType

    with tile.TileContext(nc) as tc:
        with (
            tc.tile_pool(name="inp", bufs=2) as inp,
            tc.tile_pool(name="work", bufs=1) as work,
            tc.tile_pool(name="ring", bufs=2) as ring,
            tc.tile_pool(name="ringz", bufs=1) as ringz,
            tc.tile_pool(name="scr", bufs=1) as scrp,
            tc.tile_pool(name="stage", bufs=1) as stage,
        ):
            LA = stage.tile([P, n_pairs * N_BINS], f32, tag="LA")
            LB = stage.tile([P, n_pairs * N_BINS], f32, tag="LB")
            scrD = scrp.tile([P, 2, free], f16, tag="scrD")
            scrA = scrp.tile([P, 2, free], f16, tag="scrA")
            biasB = {}
            all_act = set().union(*_ACT_SCHED)
            for t in sorted(all_act):
                bb = stage.tile([P, 1], f32, tag=f"bb{t}")
                nc.vector.memset(bb[:], -(OFF + 0.5 + t))
                biasB[t] = bb

            for pr in range(n_pairs):
                upR = ring.tile([P, 2, free], f16, tag="upR")
                # zR/u4R are produced and consumed by DVE only (in-order
                # engine), so a single buffer can't stall cross-engine.
                zR = ringz.tile([P, 2, free], f16, tag="zR")
                u4R = ringz.tile([P, 2, free], f16, tag="u4R")
                for half in range(2):
                    it = pr * 2 + half
                    r0 = (it // n_c) * P
                    c0 = (it % n_c) * free

                    xt = inp.tile([P, 4, free], f16, tag="x")
                    cm = work.tile([P, free], f16, tag="cm")
                    corr = work.tile([P, free], f16, tag="corr")
                    d = work.tile([P, free], f16, tag="d")

                    # X is laid out (conf, mask, pred, targ): land (c,m)
                    # first so cm/u' start after a half-tile DMA, and (p,t)
                    # arrives while they run. u' before corr opens the ACT
                    # probe window earlier.
                    nc.sync.dma_start(
                        xt[:, 0:2], X[r0 : r0 + P, 0:2, c0 : c0 + free]
                    )
                    nc.sync.dma_start(
                        xt[:, 2:4], X[r0 : r0 + P, 2:4, c0 : c0 + free]
                    )
                    c16 = xt[:, 0]
                    m16 = xt[:, 1]
                    p16 = xt[:, 2]
                    t16 = xt[:, 3]
                    nc.vector.tensor_tensor(cm[:], c16, m16, Alu.mult)
                    nc.vector.tensor_scalar(
                        upR[:, half], cm[:], 15.0, OFF + 0.5, Alu.mult, Alu.add
                    )
                    nc.vector.tensor_tensor(corr[:], p16, t16, Alu.is_equal)
                    nc.vector.tensor_tensor(d[:], cm[:], corr[:], Alu.subtract)
                    # u4 = (u' - 1024)*4 = 4*bin, small-magnitude B stream
                    nc.vector.tensor_scalar(
                        u4R[:, half], upR[:, half], OFF, 4.0,
                        Alu.subtract, Alu.mult,
                    )
                    nc.vector.tensor_tensor(
                        zR[:, half], u4R[:, half], d[:], Alu.add
                    )

                act_set = _ACT_SCHED[pr]
                for t in range(N_BINS):
                    colA = slice(pr * N_BINS + t, pr * N_BINS + t + 1)
                    colB = slice(pr * N_BINS + t, pr * N_BINS + t + 1)
                    # A-probe over both halves: accum = sum max(z, 4t+2)
                    nc.vector.tensor_scalar(
                        scrD[:], zR[:], 4.0 * t + 2.0, 0.0,
                        Alu.max, Alu.add, accum_out=LA[:, colA],
                    )
                    if t in act_set:
                        nc.scalar.activation(
                            scrA[:], upR[:], Act.Relu,
                            bias=biasB[t][:], accum_out=LB[:, colB],
                        )
                    else:
                        # same scalar as the A-probe: L_t = M_t - B'_t with
                        # the (4t+2)*N offsets cancelling exactly, and the
                        # u4 stream keeps f32 accumulation well-conditioned
                        nc.vector.tensor_scalar(
                            scrD[:], u4R[:], 4.0 * t + 2.0, 0.0,
                            Alu.max, Alu.add, accum_out=LB[:, colB],
                        )

                # stream this pair's accumulator columns out while the next
                # pair computes, so the final drain only waits on pair n-1
                cols = slice(pr * N_BINS, (pr + 1) * N_BINS)
                nc.sync.dma_start(outA[:, cols], LA[:, cols])
                nc.sync.dma_start(outB[:, cols], LB[:, cols])

    nc.compile()
    return nc, n_tiles


def _get_program():
    if "prog" not in _CACHE:
        _CACHE["prog"] = _build_program()
    return _CACHE["prog"]


def _combine(partA_list, partB_list, total):
    if total == 0.0:
        return np.float32(0.0)
    # Per (pair, t): DVE B'-probes share the A-probe scalar so
    # L contribution = M - B' directly; ACT relu-probes need the
    # (4t+2)*N offset removed from M and contribute -4*sum(relu).
    L = np.zeros(N_BINS, dtype=np.float64)
    n_pairs = len(_ACT_SCHED)
    n_elem_pair = float(P * 2 * FREE)
    for pa, pb in zip(partA_list, partB_list):
        pa = np.asarray(pa).astype(np.float64).reshape(P, n_pairs, N_BINS)
        pb = np.asarray(pb).astype(np.float64).reshape(P, n_pairs, N_BINS)
        for pr in range(n_pairs):
            act_set = _ACT_SCHED[pr]
            for t in range(N_BINS):
                M = pa[:, pr, t].sum()
                bv = pb[:, pr, t].sum()
                if t in act_set:
                    L[t] += M - (4.0 * t + 2.0) * n_elem_pair - 4.0 * bv
                else:
                    L[t] += M - bv
    S = L.copy()
    S[:-1] -= L[1:]
    return np.float32(np.abs(S).sum() / total)


def kernel(confidences, predictions, targets, mask):
    global LAST_EXEC_TIME_NS, LAST_RESULTS
    nc, n_tiles = _get_program()

    conf = np.asarray(confidences, dtype=np.float32)
    pred = np.asarray(predictions)
    targ = np.asarray(targets)
    msk = np.asarray(mask)
    assert conf.shape == (FULL_ROWS, COLS)

    X = np.empty((FULL_ROWS, 4, COLS), dtype=np.float16)
    X[:, 0] = conf.astype(np.float16)
    X[:, 1] = msk.astype(np.float16)
    X[:, 2] = pred.astype(np.float16)
    X[:, 3] = targ.astype(np.float16)

    in_maps = []
    for i in range(N_CORES):
        sl = slice(i * ROWS, (i + 1) * ROWS)
        in_maps.append({"x": X[sl]})

    trace = bool(int(os.environ.get("ECE_TRACE", "0")))
    res = run_bass_kernel_spmd(nc, in_maps, list(range(N_CORES)), trace=trace)
    LAST_EXEC_TIME_NS = res.exec_time_ns
    LAST_RESULTS = res

    total = float(np.asarray(msk).sum(dtype=np.int64))
    return _combine(
        [res.results[i]["pA"] for i in range(N_CORES)],
        [res.results[i]["pB"] for i in range(N_CORES)],
        total,
    )
